# revision 29
# baseline (speedup 1.0000x reference)
"""Trainium2 Bass kernel for AceStepAttention (B=2, S=2048, D=2048, H=16, KVH=4, HD=128).

Sharding: 8 cores = (batch 2) x (kv-head group 4). Each core computes, for its
batch b and kv group g: the 4 query heads [4g..4g+4) + kv head g, full
non-causal attention over S=2048, and the o_proj partial for its 512 columns
of o_w. Host sums the 4 partials per batch.

Per-core dataflow (everything PE-native layouts, transposed on host):
  hT [D,S] bf16  --matmul--> q/k/v in PSUM [tok,128*{4,1,1}] fp32
    (token chunks processed in groups of 3, contraction-chunk-major, so the
     PE fills the DMA-paced startup window)
  rms-norm factors from PSUM (ACT Square+accum, Newton-refined rsqrt)
  RoPE with head-norm-folded cos/sin tables (DVE+ACT), scaled by rq*HD^-0.5/rk
  PE-transpose -> QT/KT [HD,S] bf16; V kept natural [tok,HD] bf16
  scoresT[k,q] = KT_tile^T . QT  (PSUM) --ACT Exp--> probsT bf16 (SBUF)
  attnT[d,q] += V_tile^T . probsT; denom[*,q] += ones^T . quad-summed probsT
  attnT_sbuf = attnT * approx_recip(denom)  (DVE, bf16)
  outT[Dout,q] += owT_tile^T . attnT -> DMA (fp32); o_proj for window i is
  interleaved into the attention kc-loop of window i+1 (shares score psum)
Host: out[b] = sum_g outT(core b,g)^T.
"""

import numpy as np
import ml_dtypes

import concourse.bacc as bacc
import concourse.bass as bass
import concourse.mybir as mybir
from concourse import tile
from concourse.bass_utils import run_bass_kernel_spmd
from concourse.masks import make_identity

BF16 = mybir.dt.bfloat16
F32 = mybir.dt.float32
AF = mybir.ActivationFunctionType
ALU = mybir.AluOpType

B = 2
S = 2048
D = 2048
H = 16
KVH = 4
HD = 128
G = H // KVH          # q heads per core
N_CORES = 8
EPS = 1e-6
P = 128

INTERLEAVE_OPROJ = True


def build_program(nc, s=S, d=D, reps=1):
    """Emit the per-core SPMD program into nc (a Bacc). Returns nothing."""
    tc_n = s // P        # token chunks
    dc_n = d // P        # hidden-dim contraction chunks
    qw = min(1024, s)    # exp/psum q-window width
    nqp = s // qw
    nj = qw // 512       # 512-wide matmul sub-tiles per window
    assert tc_n % 4 == 0

    hT = nc.dram_tensor("hT", [d, s], BF16, kind="ExternalInput")
    qwT = nc.dram_tensor("qwT", [d, G * HD], BF16, kind="ExternalInput")
    kwT = nc.dram_tensor("kwT", [d, HD], BF16, kind="ExternalInput")
    vwT = nc.dram_tensor("vwT", [d, HD], BF16, kind="ExternalInput")
    owT = nc.dram_tensor("owT", [G * HD, d], BF16, kind="ExternalInput")
    cwq = nc.dram_tensor("cwq", [P, s], BF16, kind="ExternalInput")
    swq = nc.dram_tensor("swq", [P, s], BF16, kind="ExternalInput")
    cwk = nc.dram_tensor("cwk", [P, s], BF16, kind="ExternalInput")
    swk = nc.dram_tensor("swk", [P, s], BF16, kind="ExternalInput")
    outT = nc.dram_tensor("outT", [d, s], F32, kind="ExternalOutput")

    import contextlib

    with tile.TileContext(nc) as tc:
        loop_ctx = tc.For_i(0, reps, 1) if reps > 1 else contextlib.nullcontext()
        with loop_ctx, tc.tile_pool(name="persist", bufs=1) as pp:
            ident = pp.tile([P, P], BF16, tag="ident")
            make_identity(nc, ident[:, :])
            ones = pp.tile([P, P], BF16, tag="ones")
            nc.vector.memset(ones[:, :], 1.0)

            owT1 = pp.tile([P, G, d], BF16, tag="owT1", name="owT1")
            nc.scalar.dma_start(owT1[:, :, :], owT.ap().rearrange("(g p) n -> p g n", p=P))

            QT = [pp.tile([P, s], BF16, tag=f"QT{h}", name=f"QT{h}") for h in range(G)]
            KT = pp.tile([P, s], BF16, tag="KT")
            V = pp.tile([P, tc_n, HD], BF16, tag="V")
            attnT = [pp.tile([P, s], BF16, tag=f"attnT{h}", name=f"attnT{h}") for h in range(G)]

            # ---------------- Phase 1: QKV projection + norm + RoPE -------------
            with (
                tc.tile_pool(name="p1data", bufs=1) as p1,
                tc.tile_pool(name="work1", bufs=2) as wp,
                tc.tile_pool(name="psum_qkv", bufs=1, space="PSUM") as pq,
                tc.tile_pool(name="psum_tr", bufs=2, space="PSUM") as ptr,
            ):
                # phase-1-only SBUF residents, loaded with FEW BIG DMAs: each
                # dma_start costs ~0.8us of sequencer descriptor generation, so
                # coalesce 4 hT chunks / 8 weight chunks per instruction.
                # (pacing granularity stays ~2MB so the PE can start early)
                assert dc_n % 4 == 0
                n4 = dc_n // 4
                n8 = max(1, dc_n // 8)
                c8 = dc_n // n8
                hT4 = [p1.tile([P, 4, s], BF16, tag=f"hT4_{i}", name=f"hT4_{i}")
                       for i in range(n4)]
                qwT8 = [p1.tile([P, c8, G * HD], BF16, tag=f"qwT8_{i}", name=f"qwT8_{i}")
                        for i in range(n8)]
                kvwT8 = [p1.tile([P, c8, 2 * HD], BF16, tag=f"kvwT8_{i}", name=f"kvwT8_{i}")
                         for i in range(n8)]

                def hT_t(c):
                    return hT4[c // 4][:, c % 4, :]

                def qwT_t(c):
                    return qwT8[c // c8][:, c % c8, :]

                def kvwT_t(c):
                    return kvwT8[c // c8][:, c % c8, :]

                nc.sync.dma_start(qwT8[0][:, :, :],
                                  qwT.ap().rearrange("(i p) n -> p i n", p=P)[:, 0:c8, :])
                nc.sync.dma_start(kvwT8[0][:, :, 0:HD],
                                  kwT.ap().rearrange("(i p) n -> p i n", p=P)[:, 0:c8, :])
                nc.sync.dma_start(kvwT8[0][:, :, HD:2 * HD],
                                  vwT.ap().rearrange("(i p) n -> p i n", p=P)[:, 0:c8, :])
                for i in range(n4):
                    nc.sync.dma_start(hT4[i][:, :, :],
                                      hT.ap().rearrange("(i p) n -> p i n", p=P)[:, 4 * i:4 * (i + 1), :])
                for i in range(1, n8):
                    nc.sync.dma_start(qwT8[i][:, :, :],
                                      qwT.ap().rearrange("(i p) n -> p i n", p=P)[:, c8 * i:c8 * (i + 1), :])
                    nc.sync.dma_start(kvwT8[i][:, :, 0:HD],
                                      kwT.ap().rearrange("(i p) n -> p i n", p=P)[:, c8 * i:c8 * (i + 1), :])
                    nc.sync.dma_start(kvwT8[i][:, :, HD:2 * HD],
                                      vwT.ap().rearrange("(i p) n -> p i n", p=P)[:, c8 * i:c8 * (i + 1), :])
                tabs = {}
                for name, dram in (("cwq", cwq), ("swq", swq), ("cwk", cwk), ("swk", swk)):
                    t = p1.tile([P, tc_n, HD], BF16, tag=name, name=name)
                    nc.scalar.dma_start(t[:, :, :], dram.ap().rearrange("p (c d) -> p c d", d=HD))
                    tabs[name] = t

                def finish_chunk(t, ps):
                    # evacuate PSUM immediately so the PE can reuse the bank:
                    # q+k -> fp32 sbuf (DVE), v -> bf16 V tile (ACT)
                    pv = wp.tile([P, (G + 1) * HD], F32, tag="pv", name=f"pv{t}")
                    nc.vector.tensor_copy(pv[:, :], ps[:, 0:(G + 1) * HD])
                    nc.scalar.copy(V[:, t, :], ps[:, (G + 1) * HD:(G + 2) * HD])
                    ps = pv
                    # norm factors: ACT Square w/ accum -> mean; +eps; Newton rsqrt
                    ssq = wp.tile([P, 8], F32, tag="ssq", name=f"ssq{t}")
                    sqs = wp.tile([P, HD], F32, tag="sqs", name=f"sqs{t}")
                    for h in range(G + 1):
                        hs = slice(h * HD, (h + 1) * HD)
                        nc.scalar.activation(sqs[:, :], ps[:, hs], AF.Square,
                                             scale=float(HD ** -0.5),
                                             accum_out=ssq[:, h:h + 1])
                    nc.vector.tensor_scalar_add(ssq[:, 0:G + 1], ssq[:, 0:G + 1], float(EPS))
                    r0 = wp.tile([P, 8], F32, tag="r0", name=f"r0_{t}")
                    t1 = wp.tile([P, 8], F32, tag="t1", name=f"t1_{t}")
                    nc.scalar.activation(t1[:, 0:G + 1], ssq[:, 0:G + 1], AF.Sqrt)
                    nc.vector.reciprocal(r0[:, 0:G + 1], t1[:, 0:G + 1])
                    nc.vector.tensor_tensor(t1[:, 0:G + 1], r0[:, 0:G + 1], r0[:, 0:G + 1], op=ALU.mult)
                    nc.vector.tensor_tensor(t1[:, 0:G + 1], t1[:, 0:G + 1], ssq[:, 0:G + 1], op=ALU.mult)
                    nc.vector.tensor_scalar(t1[:, 0:G + 1], t1[:, 0:G + 1], -0.5, 1.5,
                                            op0=ALU.mult, op1=ALU.add)
                    nc.vector.tensor_tensor(r0[:, 0:G + 1], r0[:, 0:G + 1], t1[:, 0:G + 1], op=ALU.mult)
                    nc.vector.tensor_scalar_mul(r0[:, 0:G], r0[:, 0:G], float(HD ** -0.5))

                    # scale q/k by r; v copy on ACT
                    qs = wp.tile([P, G * HD], F32, tag="qs", name=f"qs{t}")
                    for h in range(G):
                        hs = slice(h * HD, (h + 1) * HD)
                        nc.vector.tensor_scalar_mul(qs[:, hs], ps[:, hs], r0[:, h:h + 1])
                    ks = wp.tile([P, HD], F32, tag="ks", name=f"ks{t}")
                    nc.vector.tensor_scalar_mul(ks[:, :], ps[:, G * HD:(G + 1) * HD], r0[:, G:G + 1])

                    # RoPE (rotate-half halves on ACT, products on DVE)
                    rot = wp.tile([P, G * HD], F32, tag="rot", name=f"rot{t}")
                    q3 = qs[:, :].rearrange("p (h x) -> p h x", h=G)
                    r3 = rot[:, :].rearrange("p (h x) -> p h x", h=G)
                    nc.scalar.mul(r3[:, :, 0:HD // 2], q3[:, :, HD // 2:HD], -1.0)
                    nc.scalar.copy(r3[:, :, HD // 2:HD], q3[:, :, 0:HD // 2])
                    qf = wp.tile([P, G * HD], BF16, tag="qf", name=f"qf{t}", bufs=4)
                    for h in range(G):
                        hs = slice(h * HD, (h + 1) * HD)
                        nc.vector.tensor_tensor(rot[:, hs], rot[:, hs], tabs["swq"][:, t, :], op=ALU.mult)
                        nc.vector.tensor_tensor(qs[:, hs], qs[:, hs], tabs["cwq"][:, t, :], op=ALU.mult)
                        nc.vector.tensor_tensor(qf[:, hs], rot[:, hs], qs[:, hs], op=ALU.add)
                    krot = wp.tile([P, HD], F32, tag="krot", name=f"krot{t}")
                    nc.scalar.mul(krot[:, 0:HD // 2], ks[:, HD // 2:HD], -1.0)
                    nc.scalar.copy(krot[:, HD // 2:HD], ks[:, 0:HD // 2])
                    kf = wp.tile([P, HD], BF16, tag="kf", name=f"kf{t}", bufs=4)
                    nc.vector.tensor_tensor(krot[:, :], krot[:, :], tabs["swk"][:, t, :], op=ALU.mult)
                    nc.vector.tensor_tensor(ks[:, :], ks[:, :], tabs["cwk"][:, t, :], op=ALU.mult)
                    nc.vector.tensor_tensor(kf[:, :], krot[:, :], ks[:, :], op=ALU.add)

                    # transpose q heads + k into [HD, S] layout (one psum bank);
                    # deferred one group so the PE isn't gated on the DVE chain
                    def do_transpose(t=t, qf=qf, kf=kf):
                        ts_ = slice(t * P, (t + 1) * P)
                        pst = ptr.tile([P, (G + 1) * HD], BF16, tag="pst", name=f"pst{t}")
                        for h in range(G):
                            hs = slice(h * HD, (h + 1) * HD)
                            nc.tensor.transpose(pst[:, hs], qf[:, hs], ident[:, :])
                        nc.tensor.transpose(pst[:, G * HD:(G + 1) * HD], kf[:, :], ident[:, :])
                        for h in range(G):
                            hs = slice(h * HD, (h + 1) * HD)
                            nc.vector.tensor_copy(QT[h][:, ts_], pst[:, hs])
                        nc.vector.tensor_copy(KT[:, ts_], pst[:, G * HD:(G + 1) * HD])
                    return do_transpose

                # token chunks in groups of 3, contraction-chunk-major inside a
                # group, so early matmuls keep pace with the hT DMA stream
                pending_tr = []
                t0 = 0
                while t0 < tc_n:
                    grp = list(range(t0, min(t0 + 3, tc_n)))
                    t0 += len(grp)
                    pss = [pq.tile([P, (G + 2) * HD], F32, tag=f"ps{i}",
                                   name=f"ps{i}_{grp[0]}") for i in range(len(grp))]
                    for c in range(dc_n):
                        st = dict(start=(c == 0), stop=(c == dc_n - 1))
                        for i, t in enumerate(grp):
                            ts_ = slice(t * P, (t + 1) * P)
                            nc.tensor.matmul(pss[i][:, 0:G * HD],
                                             lhsT=hT4[c // 4][:, c % 4, ts_],
                                             rhs=qwT_t(c), **st)
                            nc.tensor.matmul(pss[i][:, G * HD:(G + 2) * HD],
                                             lhsT=hT4[c // 4][:, c % 4, ts_],
                                             rhs=kvwT_t(c), **st)
                    for fn in pending_tr:
                        fn()
                    pending_tr = [finish_chunk(t, pss[i]) for i, t in enumerate(grp)]
                for fn in pending_tr:
                    fn()

            # ---------------- Phase 2 (+interleaved Phase 3) ---------------------
            with (
                tc.tile_pool(name="psum_sc", bufs=3, space="PSUM") as psc,
                tc.tile_pool(name="psum_at", bufs=1, space="PSUM") as pat,
                tc.tile_pool(name="work2", bufs=3) as wp2,
                tc.tile_pool(name="work3", bufs=4) as wp3,
            ):
                def oproj_tile(oc, qp):
                    os_ = slice(oc * P, (oc + 1) * P)
                    ob = wp3.tile([P, qw], F32, tag="ob", name=f"ob{oc}_{qp}", bufs=3)
                    for j in range(nj):
                        qc = qp * nj + j
                        qs_ = slice(qc * 512, (qc + 1) * 512)
                        ot = psc.tile([P, 512], F32, tag="sc", name=f"ot{oc}_{qc}")
                        for g in range(G):
                            nc.tensor.matmul(ot[:, :], lhsT=owT1[:, g, os_],
                                             rhs=attnT[g][:, qs_],
                                             start=(g == 0), stop=(g == G - 1))
                        nc.vector.tensor_copy(ob[:, j * 512:(j + 1) * 512], ot[:, :])
                    nc.sync.dma_start(outT.ap()[os_, qp * qw:(qp + 1) * qw], ob[:, :])

                pending = []
                nquad = tc_n // 4
                for qp in range(nqp):
                    for h in range(G):
                        at = pat.tile([P, qw], F32, tag="at", name=f"at{h}_{qp}")
                        pb0 = pbq = None
                        quads = []
                        for kc in range(tc_n):
                            ks_ = slice(kc * P, (kc + 1) * P)
                            sc = psc.tile([P, qw], F32, tag="sc", name=f"sc{h}_{qp}_{kc}")
                            for j in range(nj):
                                qs_ = slice(qp * qw + j * 512, qp * qw + (j + 1) * 512)
                                nc.tensor.matmul(sc[:, j * 512:(j + 1) * 512],
                                                 lhsT=KT[:, ks_], rhs=QT[h][:, qs_],
                                                 start=True, stop=True)
                            pb = wp2.tile([P, qw], BF16, tag="pb", name=f"pb{h}_{qp}_{kc}")
                            nc.scalar.activation(pb[:, :], sc[:, :], AF.Exp)
                            st = dict(start=(kc == 0), stop=(kc == tc_n - 1))
                            for j in range(nj):
                                js = slice(j * 512, (j + 1) * 512)
                                nc.tensor.matmul(at[:, js], lhsT=V[:, kc, :], rhs=pb[:, js], **st)
                            # denominator: running quad-sums of probs on DVE (bf16)
                            iq = kc % 4
                            if iq == 0:
                                pb0 = pb
                            elif iq == 1:
                                pbq = wp2.tile([P, qw], BF16, tag=f"pbq{kc // 4}",
                                               name=f"pbq{h}_{qp}_{kc}", bufs=1)
                                nc.vector.tensor_tensor(pbq[:, :], pb0[:, :], pb[:, :], op=ALU.add)
                            else:
                                nc.vector.tensor_tensor(pbq[:, :], pbq[:, :], pb[:, :], op=ALU.add)
                            if iq == 3:
                                quads.append(pbq)
                            # interleave o_proj of the previous q-window
                            if INTERLEAVE_OPROJ and pending and kc % 4 == 2:
                                oproj_tile(*pending.pop(0))
                        # fp32 tree over the quad sums, bf16 total, then a single
                        # transient pair of ones-matmuls for the denominator
                        while len(quads) > 1:
                            nxt = []
                            for i in range(0, len(quads) - 1, 2):
                                dt_ = BF16 if len(quads) == 2 else F32
                                tsum = wp2.tile([P, qw], dt_, tag=f"pbt{len(quads)}_{i}",
                                                name=f"pbt{h}_{qp}_{len(quads)}_{i}", bufs=1)
                                nc.vector.tensor_tensor(tsum[:, :], quads[i][:, :],
                                                        quads[i + 1][:, :], op=ALU.add)
                                nxt.append(tsum)
                            if len(quads) % 2:
                                nxt.append(quads[-1])
                            quads = nxt
                        dn = psc.tile([P, qw], F32, tag="sc", name=f"dn{h}_{qp}")
                        for j in range(nj):
                            js = slice(j * 512, (j + 1) * 512)
                            nc.tensor.matmul(dn[:, js], lhsT=ones[:, :], rhs=quads[0][:, js],
                                             start=True, stop=True)
                        inv = wp2.tile([P, qw], F32, tag="inv", name=f"inv{h}_{qp}", bufs=2)
                        for j in range(nj):
                            js = slice(j * 512, (j + 1) * 512)
                            ws = slice(qp * qw + j * 512, qp * qw + (j + 1) * 512)
                            nc.vector.reciprocal_approx_fast(inv[:, js], dn[:, js])
                            nc.vector.tensor_tensor(attnT[h][:, ws], at[:, js], inv[:, js], op=ALU.mult)
                    for oc in range(dc_n):
                        pending.append((oc, qp))
                for oc_qc in pending:
                    oproj_tile(*oc_qc)


_COMPILED = {}


def _get_compiled(num_devices=N_CORES):
    key = num_devices
    if key not in _COMPILED:
        nc = bacc.Bacc("TRN2", target_bir_lowering=False, debug=False,
                       num_devices=num_devices)
        build_program(nc)
        nc.compile()
        _COMPILED[key] = nc
    return _COMPILED[key]


def _bf16(x):
    return np.ascontiguousarray(x).astype(ml_dtypes.bfloat16)


def prep_in_maps(hidden_states, cos, sin, q_w, k_w, v_w, o_w, q_norm_w, k_norm_w):
    """Shard + pre-transpose + cast the full inputs into 8 per-core maps."""
    hidden_states = np.asarray(hidden_states, np.float32)
    cos = np.asarray(cos, np.float32)
    sin = np.asarray(sin, np.float32)
    q_w = np.asarray(q_w, np.float32)
    k_w = np.asarray(k_w, np.float32)
    v_w = np.asarray(v_w, np.float32)
    o_w = np.asarray(o_w, np.float32)
    q_norm_w = np.asarray(q_norm_w, np.float32)
    k_norm_w = np.asarray(k_norm_w, np.float32)

    # norm weights folded into the rope tables (see module docstring)
    qn_rot = np.concatenate([q_norm_w[HD // 2:], q_norm_w[:HD // 2]])
    kn_rot = np.concatenate([k_norm_w[HD // 2:], k_norm_w[:HD // 2]])

    def _tab(x):
        # [S, HD] -> [128, S] chunk-major: row p holds [chunk0, chunk1, ...]
        return _bf16(x.reshape(S // P, P, HD).transpose(1, 0, 2).reshape(P, S))

    cwq = _tab(cos * q_norm_w[None, :])
    swq = _tab(sin * qn_rot[None, :])
    cwk = _tab(cos * k_norm_w[None, :])
    swk = _tab(sin * kn_rot[None, :])

    in_maps = []
    for c in range(N_CORES):
        b, g = c // KVH, c % KVH
        in_maps.append({
            "hT": _bf16(hidden_states[b].T),
            "qwT": _bf16(q_w[g * G * HD:(g + 1) * G * HD, :].T),
            "kwT": _bf16(k_w[g * HD:(g + 1) * HD, :].T),
            "vwT": _bf16(v_w[g * HD:(g + 1) * HD, :].T),
            "owT": _bf16(o_w[:, g * G * HD:(g + 1) * G * HD].T),
            "cwq": cwq, "swq": swq, "cwk": cwk, "swk": swk,
        })
    return in_maps


def kernel(**inputs):
    nc = _get_compiled()
    in_maps = prep_in_maps(**inputs)
    res = run_bass_kernel_spmd(nc, in_maps, core_ids=list(range(N_CORES)))
    out = np.zeros((B, S, D), np.float32)
    for c in range(N_CORES):
        out[c // KVH] += res.results[c]["outT"].T
    return out
